# revision 1
# baseline (speedup 1.0000x reference)
"""LIF activation (hard-reset leaky integrate-and-fire) on 8 Trainium2 cores.

Math (per lane, per step t):
    u_t   = x_t + z_{t-1}
    Vm_t  = relu(u_t)
    keep  = 1{Vm_t < 1}  == 1{u_t < 1}
    z_t   = (1 - w_leak) * Vm_t * keep     (carried state, pre-scaled)
    spike = 1{u_t > 1}                     (strict >, the output)

Device per-step chain (all exact f32, DVE):
    u  = x + z                        tensor_tensor add
    g  = (u >= 1)                     tensor_scalar is_ge
    m' = min(u, 1) - g                scalar_tensor_tensor
    z  = max(m', 0) * W1              scalar_tensor_tensor (relu fused)
    spike = (u > 1)                   gpsimd tensor_scalar is_gt (off-path)

Sharding: time is split across the 8 cores (125 output steps each) with a
W-step speculative warmup from z=0. Hard resets make any starting state
collapse: if within the warmup window a lane sees x_t >= 1 (forced reset for
any state, u = x+z >= 1) or x_t <= -(1-w_leak) (forced relu clamp, z < 1-w),
the warmed-up state is provably bit-exact. Lanes with no such certificate in
some window (a few hundred of 65536) are recomputed exactly on the host and
patched in, so the result is exact for any input.
"""
import numpy as np
import sys

for _p in ("/opt/trn_rl_repo",):
    if _p not in sys.path:
        sys.path.append(_p)

import concourse.bass as bass
import concourse.mybir as mybir
from concourse.tile import TileContext
from concourse import bass_utils

F32 = mybir.dt.float32
OP = mybir.AluOpType

B, T, C = 128, 1000, 512
NCORES = 8
WARM = 16                 # speculative warmup steps
L = T // NCORES           # 125 output steps per core
T_IN = L + WARM           # 141 compute steps per core
CHUNK = 16                # time steps per DMA chunk

TRACE = False             # set True (e.g. from test.py) to capture NTFF profile
LAST_RESULTS = None       # BassKernelResults of the last run, for profiling


def out_chunk_sizes(l_out, chunk):
    return [min(chunk, l_out - o) for o in range(0, l_out, chunk)]


def lif_body(tc, out_aps, x_ap, w1_ap, t_in=T_IN, warm=WARM, chunk=CHUNK):
    """Emit the per-core LIF program.

    out_aps: list of [B, osz_i, C] f32 DRAM tensors, one per output chunk
    x_ap: [B, t_in, C] f32 DRAM
    w1_ap:  [128, C] f32 DRAM (1 - w_leak, pre-broadcast over partitions)
    """
    nc = tc.nc
    l_out = t_in - warm
    with tc.tile_pool(name="const", bufs=1) as constp, \
         tc.tile_pool(name="state", bufs=1) as statep, \
         tc.tile_pool(name="xin", bufs=3) as xinp, \
         tc.tile_pool(name="outs", bufs=2) as outp:
        w1t = constp.tile([128, C], F32)
        nc.sync.dma_start(out=w1t, in_=w1_ap)
        z = statep.tile([128, C], F32)
        nc.vector.memset(z, 0.0)
        # persistent work tiles: all hazards on these are DVE-internal
        # (in-order), so no slot-rotation semaphores are generated
        u = statep.tile([128, C], F32, tag="u")
        g = statep.tile([128, C], F32, tag="g")
        mp = statep.tile([128, C], F32, tag="mp")
        # 1-element scratch: "touch" ops read freshly-DMA'd tiles here so the
        # DMA-completion wait lands on the touch, not on a compute op (the TT
        # ISA struct only has room for one sync wait, and the serial RAW chain
        # already consumes it on every compute instruction).
        touch_w = statep.tile([128, 1], F32, tag="touch_w")
        nc.vector.tensor_copy(out=touch_w, in_=w1t[:, :1])

        xin_tiles = []
        xin = None
        csz = 0
        out_tile = None
        o0 = osz = 0
        oc = 0
        for s in range(t_in):
            ic, off = divmod(s, chunk)
            if off == 0:
                csz = min(chunk, t_in - s)
                xin = xinp.tile([128, csz, C], F32, tag="xin")
                xin_tiles.append(xin)
                nc.sync.dma_start(out=xin, in_=x_ap[:, s:s + csz, :])
                tch = statep.tile([128, 1], F32, tag=f"touch{ic}")
                nc.vector.tensor_copy(out=tch, in_=xin[:, 0, :1])

            nc.vector.tensor_add(out=u, in0=xin[:, off, :], in1=z)
            nc.vector.tensor_scalar(out=g, in0=u, scalar1=1.0, scalar2=None,
                                    op0=OP.is_ge)
            nc.vector.scalar_tensor_tensor(out=mp, in0=u, scalar=1.0, in1=g,
                                           op0=OP.min, op1=OP.subtract)
            nc.vector.scalar_tensor_tensor(out=z, in0=mp, scalar=0.0, in1=w1t,
                                           op0=OP.max, op1=OP.mult)
            if off == csz - 1 and s < t_in - 1:
                # release marker: after the chunk's last read, a 4-byte DVE
                # write makes DVE the slot's most-recent writer, so the next
                # refill DMA needs only a single DVE wait (which transitively
                # implies the old DMA completed — its touch waited on it).
                nc.vector.memset(xin[:, 0, :1], 0.0)

            o = s - warm
            if o >= 0:
                oo = o - o0
                if oo == 0:
                    osz = min(chunk, l_out - o)
                    out_tile = outp.tile([128, osz, C], F32, tag="out")
                    # absorb the out-slot-release (DMA read done) wait
                    nc.vector.memset(out_tile[:, 0, :1], 0.0)
                nc.vector.tensor_scalar(out=out_tile[:, oo, :], in0=u,
                                        scalar1=1.0, scalar2=None, op0=OP.is_gt)
                if oo == osz - 1:
                    # separate DRAM tensor per out-chunk: avoids a false WAW
                    # between consecutive stores to one output tensor
                    nc.sync.dma_start(out=out_aps[oc], in_=out_tile)
                    o0 += osz
                    oc += 1


def _legalize_waits(nc):
    """Walrus accepts at most one sync wait on compute/DMA ISA structs.
    Split extra waits onto standalone EventSemaphore instructions inserted
    just before, on the same engine queue (identical blocking semantics)."""
    import bass_rust
    skip = ("InstEventSemaphore",)
    for f in nc.m.functions:
        for bb in f.blocks:
            insts = bb.instructions
            k = 0
            while k < len(insts):
                i = insts[k]
                si = i.sync_info
                if (si is not None and si.on_wait and len(si.on_wait) > 1
                        and type(i).__name__ not in skip):
                    waits = list(si.on_wait)
                    for j, w in enumerate(waits[:-1]):
                        ev = mybir.InstEventSemaphore(
                            name=f"{i.name}-evw{j}",
                            engine=i.engine,
                            ins=[], outs=[],
                            sync_info=bass_rust.SyncInfo(
                                on_wait=[w], on_update=[]),
                        )
                        insts.insert(k, ev)
                        k += 1
                    i.sync_info = bass_rust.SyncInfo(
                        on_wait=[waits[-1]], on_update=si.on_update)
                k += 1


def build(t_in=T_IN, warm=WARM, chunk=CHUNK):
    nc = bass.Bass("TRN2", target_bir_lowering=False, debug=False,
                   enable_asserts=False, num_devices=NCORES)
    x_d = nc.dram_tensor("x_local", [B, t_in, C], F32, kind="ExternalInput")
    w1_d = nc.dram_tensor("w1b", [128, C], F32, kind="ExternalInput")
    out_ds = [
        nc.dram_tensor(f"spikes{i}", [B, osz, C], F32, kind="ExternalOutput")
        for i, osz in enumerate(out_chunk_sizes(t_in - warm, chunk))
    ]
    with TileContext(nc) as tc:
        lif_body(tc, [d[:] for d in out_ds], x_d[:], w1_d[:], t_in, warm, chunk)
    _legalize_waits(nc)
    return nc


def _host_repair(out, x, w1):
    """Exactly recompute lanes whose warmup windows lack a reset/clamp
    certificate at some core boundary, and patch them into `out`."""
    missing = np.zeros((B, C), bool)
    for k in range(1, NCORES):
        t0 = k * L
        win = x[:, t0 - WARM:t0, :]
        cert = ((win >= np.float32(1.0)) |
                (win <= -w1[None, None, :])).any(axis=1)
        missing |= ~cert
    if not missing.any():
        return 0
    bb, cc = np.nonzero(missing)
    xs = x[bb, :, cc]                     # [R, T]
    a = w1[cc]                            # [R]
    zz = np.zeros(len(bb), np.float32)
    one = np.float32(1.0)
    zero = np.float32(0.0)
    sp = np.empty((len(bb), T), np.float32)
    for t in range(T):
        u = (xs[:, t] + zz).astype(np.float32)
        g = (u >= one).astype(np.float32)
        mp = (np.minimum(u, one) - g).astype(np.float32)
        zz = (np.maximum(mp, zero) * a).astype(np.float32)
        sp[:, t] = (u > one).astype(np.float32)
    out[bb, :, cc] = sp
    return len(bb)


def kernel(x, w_leak):
    global LAST_RESULTS
    x = np.ascontiguousarray(np.asarray(x), dtype=np.float32)
    w_leak = np.ascontiguousarray(np.asarray(w_leak), dtype=np.float32)
    w1 = (np.float32(1.0) - w_leak).astype(np.float32)       # [C]
    w1b = np.ascontiguousarray(np.broadcast_to(w1[None, :], (128, C)),
                               dtype=np.float32)

    in_maps = []
    for k in range(NCORES):
        t0 = k * L
        if k == 0:
            xs = np.concatenate(
                [np.zeros((B, WARM, C), np.float32), x[:, :L, :]], axis=1)
        else:
            xs = x[:, t0 - WARM:t0 + L, :]
        in_maps.append({"x_local": np.ascontiguousarray(xs), "w1b": w1b})

    nc = build()
    res = bass_utils.run_bass_kernel_spmd(
        nc, in_maps, core_ids=list(range(NCORES)), trace=TRACE)
    LAST_RESULTS = res
    nchunks = len(out_chunk_sizes(L, CHUNK))
    out = np.concatenate(
        [res.results[k][f"spikes{i}"]
         for k in range(NCORES) for i in range(nchunks)], axis=1)
    _host_repair(out, x, w1)
    return out



# revision 2
# speedup vs baseline: 1.0504x; 1.0504x over previous
"""LIF activation on 8 Trainium2 cores — pool-less packed-DVE kernel.

Per core the 125 output steps split into two consecutive sub-shards
(A: 63 steps, B: 62 steps) packed side by side in the free dimension, so
every chain instruction runs on [128, 2C] and the per-instruction overhead
is amortized over two time steps. GpSimd is left idle on purpose: measured
on hardware, a concurrent gpsimd chain contends for the shared SBUF ports
and slows DVE ops ~3x, which costs more than gpsimd contributes.

Chain per round (advances both sub-shards one step):
    u = x + z                    tensor_tensor add       (DVE)
    m = (u is_lt 1) mult u       scalar_tensor_tensor    (DVE)
    z = (m max 0) mult w1t2      scalar_tensor_tensor    (DVE)
    spikes: ACT Sign(u-1) -> i8, one op per round-pair (host maps >0)

Each sub-shard warms up WARM steps from z=0: hard resets collapse any
starting state, so if a lane's warmup window contains x_t >= 1 (forced
reset) or x_t <= -(1-w_leak) (forced relu clamp), the warmed state is
bit-exact. Lanes missing a certificate at some boundary are recomputed
exactly on the host. Sub-shard B runs one dummy tail round (its half of
the final pair) whose output is discarded on the host.
"""
import numpy as np
import sys

for _p in ("/opt/trn_rl_repo",):
    if _p not in sys.path:
        sys.path.append(_p)

import concourse.bass as bass
import concourse.mybir as mybir
from concourse.tile import TileContext
from concourse import bass_utils

F32 = mybir.dt.float32
I8 = mybir.dt.int8
OP = mybir.AluOpType
AF = mybir.ActivationFunctionType

B, T, C = 128, 1000, 512
NCORES = 8
WARM = 12                 # speculative warmup steps per sub-shard
L = T // NCORES           # 125 output steps per core
LA = 63                   # sub-shard A out steps; B covers 62 (+1 dummy)
CHUNK = 12                # output chunk (rounds)
INCHUNK = 6               # input-DMA chunk (rounds)
FIRSTCHUNK = 4            # small first input chunk: fast engine start
NPAIR = 4                 # u pair-ring depth (2*NPAIR rounds of ACT slack)
XBUFS = 5                 # xin tile-pool buffers

TRACE = False
LAST_RESULTS = None


def out_chunk_sizes(l_out, chunk=CHUNK):
    return [min(chunk, l_out - o) for o in range(0, l_out, chunk)]


def in_chunk_plan(t_in):
    plan, s = [], 0
    while s < t_in:
        sz = min(FIRSTCHUNK if s == 0 else INCHUNK, t_in - s)
        plan.append((s, sz))
        s += sz
    return plan


def lif_body(tc, out_aps, x_ap, w12_ap):
    nc = tc.nc
    t_in = LA + WARM
    with tc.tile_pool(name="const", bufs=1) as constp, \
         tc.tile_pool(name="state", bufs=1) as statep, \
         tc.tile_pool(name="xin", bufs=XBUFS) as xinp, \
         tc.tile_pool(name="outs", bufs=2) as outp:
        w1t2 = constp.tile([128, 2 * C], F32)
        nc.sync.dma_start(out=w1t2, in_=w12_ap)
        tchw = statep.tile([128, 1], F32, tag="tchw")
        nc.vector.tensor_copy(out=tchw, in_=w1t2[:, :1])
        negone = constp.tile([128, 1], F32, tag="negone")
        nc.vector.memset(negone, -1.0)

        z = statep.tile([128, 2 * C], F32, tag="z")
        nc.vector.memset(z, 0.0)
        upair = []
        for i in range(NPAIR):
            up = statep.tile([128, 2, 2 * C], F32, tag=f"up{i}")
            upair.append(up)
        m = statep.tile([128, 2 * C], F32, tag="m")

        plan = in_chunk_plan(t_in)
        ci = -1
        c0 = csz = 0
        xin = None
        out_tile = None
        o0 = osz = oc = 0
        for s in range(t_in):
            if ci + 1 < len(plan) and s == plan[ci + 1][0]:
                ci += 1
                c0, csz = plan[ci]
                xin = xinp.tile([128, csz, 2 * C], F32, tag="xin")
                nc.sync.dma_start(out=xin, in_=x_ap[:, s:s + csz, :])
                # touch: the DMA-completion wait lands here, off the chain
                tch = xinp.tile([128, 1], F32, tag="tch")
                nc.vector.tensor_copy(out=tch, in_=xin[:, 0, :1])
            off = s - c0
            o = s - WARM
            if o >= 0:
                u = upair[(o // 2) % NPAIR][:, o & 1, :]
            else:
                u = upair[0][:, s & 1, :]
            nc.vector.tensor_add(out=u, in0=xin[:, off, :], in1=z)
            nc.vector.scalar_tensor_tensor(out=m, in0=u, scalar=1.0, in1=u,
                                           op0=OP.is_lt, op1=OP.mult)
            nc.vector.scalar_tensor_tensor(out=z, in0=m, scalar=0.0,
                                           in1=w1t2, op0=OP.max,
                                           op1=OP.mult)
            if off == csz - 1 and s < t_in - 1:
                # release marker: DVE becomes the slot's last writer so the
                # refill DMA needs only a single engine-sem wait
                nc.vector.memset(xin[:, 0, :1], 0.0)

            if o >= 0 and (o % 2 == 1 or o == LA - 1):
                op0 = o - (o % 2)
                npair = min(2, LA - op0)
                oo = op0 - o0
                if oo == 0:
                    osz = min(CHUNK, LA - op0)
                    out_tile = outp.tile([128, osz, 2 * C], I8, tag="out")
                    # absorb the out-slot-release (DMA read done) wait
                    nc.scalar.memzero(out_tile[:, 0, 0:4])
                srcp = upair[(op0 // 2) % NPAIR]
                if npair == 2:
                    nc.scalar.activation(out=out_tile[:, oo:oo + 2, :],
                                         in_=srcp, func=AF.Sign,
                                         bias=negone, scale=1.0)
                else:
                    nc.scalar.activation(out=out_tile[:, oo, :],
                                         in_=srcp[:, 0, :], func=AF.Sign,
                                         bias=negone, scale=1.0)
                if oo + npair == osz:
                    # ACT-issued: keeps the SP queue free for input loads
                    nc.scalar.dma_start(out=out_aps[oc], in_=out_tile)
                    o0 += osz
                    oc += 1


def _legalize_waits(nc):
    """Walrus accepts at most one sync wait on compute/DMA ISA structs.
    Split extra waits onto standalone EventSemaphore instructions inserted
    just before, on the same engine queue (identical blocking semantics)."""
    import bass_rust
    skip = ("InstEventSemaphore",)
    for f in nc.m.functions:
        for bb in f.blocks:
            insts = bb.instructions
            k = 0
            while k < len(insts):
                i = insts[k]
                si = i.sync_info
                if (si is not None and si.on_wait and len(si.on_wait) > 1
                        and type(i).__name__ not in skip):
                    waits = list(si.on_wait)
                    for j, w in enumerate(waits[:-1]):
                        ev = mybir.InstEventSemaphore(
                            name=f"{i.name}-evw{j}",
                            engine=i.engine,
                            ins=[], outs=[],
                            sync_info=bass_rust.SyncInfo(
                                on_wait=[w], on_update=[]),
                        )
                        insts.insert(k, ev)
                        k += 1
                    i.sync_info = bass_rust.SyncInfo(
                        on_wait=[waits[-1]], on_update=si.on_update)
                k += 1


def build():
    nc = bass.Bass("TRN2", target_bir_lowering=False, debug=False,
                   enable_asserts=False, num_devices=NCORES)
    xd = nc.dram_tensor("x_dve", [B, WARM + LA, 2 * C], F32,
                        kind="ExternalInput")
    w12_d = nc.dram_tensor("w1b2", [128, 2 * C], F32, kind="ExternalInput")
    out_d = [
        nc.dram_tensor(f"spikes{i}", [B, osz, 2 * C], I8,
                       kind="ExternalOutput")
        for i, osz in enumerate(out_chunk_sizes(LA))
    ]
    with TileContext(nc) as tc:
        lif_body(tc, [d[:] for d in out_d], xd[:], w12_d[:])
    _legalize_waits(nc)
    return nc


def _host_repair(out, x, w1):
    """Exactly recompute lanes whose warmup windows lack a reset/clamp
    certificate at some sub-shard boundary, and patch them into `out`."""
    bounds = []
    for k in range(NCORES):
        if k > 0:
            bounds.append(k * L)
        bounds.append(k * L + LA)
    missing = np.zeros((B, C), bool)
    for t0 in bounds:
        win = x[:, t0 - WARM:t0, :]
        cert = ((win >= np.float32(1.0)) |
                (win <= -w1[None, None, :])).any(axis=1)
        missing |= ~cert
    if not missing.any():
        return 0
    bb, cc = np.nonzero(missing)
    xs = x[bb, :, cc]
    a = w1[cc]
    zz = np.zeros(len(bb), np.float32)
    one = np.float32(1.0)
    zero = np.float32(0.0)
    sp = np.empty((len(bb), T), np.float32)
    for t in range(T):
        u = (xs[:, t] + zz).astype(np.float32)
        mm = ((u < one).astype(np.float32) * u).astype(np.float32)
        zz = (np.maximum(mm, zero) * a).astype(np.float32)
        sp[:, t] = (u > one).astype(np.float32)
    out[bb, :, cc] = sp
    return len(bb)


def kernel(x, w_leak):
    global LAST_RESULTS
    x = np.ascontiguousarray(np.asarray(x), dtype=np.float32)
    w_leak = np.ascontiguousarray(np.asarray(w_leak), dtype=np.float32)
    w1 = (np.float32(1.0) - w_leak).astype(np.float32)
    w1b2 = np.ascontiguousarray(np.broadcast_to(
        np.concatenate([w1, w1])[None, :], (128, 2 * C)), dtype=np.float32)

    # xw: x with a global WARM-zero prefix so every warmup window indexes
    # uniformly (core 0 sub-shard A starts exactly from z=0). One extra
    # zero tail step feeds sub-shard B's dummy final round.
    nsteps = WARM + LA
    xw = np.concatenate([np.zeros((B, WARM, C), np.float32), x,
                         np.zeros((B, 1, C), np.float32)], axis=1)
    in_maps = []
    for k in range(NCORES):
        t0 = k * L
        xa = xw[:, t0:t0 + nsteps, :]
        xb = xw[:, t0 + LA:t0 + LA + nsteps, :]
        xd = np.ascontiguousarray(
            np.stack([xa, xb], axis=2).reshape(B, nsteps, 2 * C))
        in_maps.append({"x_dve": xd, "w1b2": w1b2})

    nc = build()
    res = bass_utils.run_bass_kernel_spmd(
        nc, in_maps, core_ids=list(range(NCORES)), trace=TRACE)
    LAST_RESULTS = res
    nch = len(out_chunk_sizes(LA))
    out = np.empty((B, T, C), np.float32)
    for k in range(NCORES):
        t0 = k * L
        da = np.concatenate(
            [res.results[k][f"spikes{i}"] for i in range(nch)],
            axis=1).reshape(B, LA, 2, C)
        out[:, t0:t0 + LA, :] = da[:, :, 0, :] > 0
        out[:, t0 + LA:t0 + L, :] = da[:, :LA - 1, 1, :] > 0
    _host_repair(out, x, w1)
    return out


# revision 3
# speedup vs baseline: 1.0512x; 1.0007x over previous
"""LIF activation on 8 Trainium2 cores — pool-less packed-DVE kernel.

Per core the 125 output steps split into two consecutive sub-shards
(A: 63 steps, B: 62 steps) packed side by side in the free dimension, so
every chain instruction runs on [128, 2C] and the per-instruction overhead
is amortized over two time steps. GpSimd is left idle on purpose: measured
on hardware, a concurrent gpsimd chain contends for the shared SBUF ports
and slows DVE ops ~3x, which costs more than gpsimd contributes.

Chain per round (advances both sub-shards one step):
    u = x + z                    tensor_tensor add       (DVE)
    m = (u is_lt 1) mult u       scalar_tensor_tensor    (DVE)
    z = (m max 0) mult w1t2      scalar_tensor_tensor    (DVE)
    spikes: ACT Sign(u-1) -> i8, one op per round-pair (host maps >0)

Each sub-shard warms up WARM steps from z=0: hard resets collapse any
starting state, so if a lane's warmup window contains x_t >= 1 (forced
reset) or x_t <= -(1-w_leak) (forced relu clamp), the warmed state is
bit-exact. Lanes missing a certificate at some boundary are recomputed
exactly on the host. Sub-shard B runs one dummy tail round (its half of
the final pair) whose output is discarded on the host.
"""
import numpy as np
import sys

for _p in ("/opt/trn_rl_repo",):
    if _p not in sys.path:
        sys.path.append(_p)

import concourse.bass as bass
import concourse.mybir as mybir
from concourse.tile import TileContext
from concourse import bass_utils

F32 = mybir.dt.float32
I8 = mybir.dt.int8
OP = mybir.AluOpType
AF = mybir.ActivationFunctionType

B, T, C = 128, 1000, 512
NCORES = 8
WARM = 8                  # speculative warmup steps per sub-shard
L = T // NCORES           # 125 output steps per core
LA = 63                   # sub-shard A out steps; B covers 62 (+1 dummy)
CHUNK = 12                # output chunk (rounds)
INCHUNK = 6               # input-DMA chunk (rounds)
FIRSTCHUNK = 4            # small first input chunk: fast engine start
NPAIR = 4                 # u pair-ring depth (2*NPAIR rounds of ACT slack)
XBUFS = 5                 # xin tile-pool buffers

TRACE = False
LAST_RESULTS = None


def out_chunk_sizes(l_out, chunk=CHUNK):
    return [min(chunk, l_out - o) for o in range(0, l_out, chunk)]


def in_chunk_plan(t_in):
    plan, s = [], 0
    while s < t_in:
        sz = min(FIRSTCHUNK if s == 0 else INCHUNK, t_in - s)
        plan.append((s, sz))
        s += sz
    return plan


def lif_body(tc, out_aps, x_ap, w12_ap):
    nc = tc.nc
    t_in = LA + WARM
    with tc.tile_pool(name="const", bufs=1) as constp, \
         tc.tile_pool(name="state", bufs=1) as statep, \
         tc.tile_pool(name="xin", bufs=XBUFS) as xinp, \
         tc.tile_pool(name="outs", bufs=2) as outp:
        w1t2 = constp.tile([128, 2 * C], F32)
        nc.sync.dma_start(out=w1t2, in_=w12_ap)
        tchw = statep.tile([128, 1], F32, tag="tchw")
        nc.vector.tensor_copy(out=tchw, in_=w1t2[:, :1])
        negone = constp.tile([128, 1], F32, tag="negone")
        nc.vector.memset(negone, -1.0)

        z = statep.tile([128, 2 * C], F32, tag="z")
        nc.vector.memset(z, 0.0)
        upair = []
        for i in range(NPAIR):
            up = statep.tile([128, 2, 2 * C], F32, tag=f"up{i}")
            upair.append(up)
        m = statep.tile([128, 2 * C], F32, tag="m")

        plan = in_chunk_plan(t_in)
        ci = -1
        c0 = csz = 0
        xin = None
        out_tile = None
        o0 = osz = oc = 0
        for s in range(t_in):
            if ci + 1 < len(plan) and s == plan[ci + 1][0]:
                ci += 1
                c0, csz = plan[ci]
                xin = xinp.tile([128, csz, 2 * C], F32, tag="xin")
                nc.sync.dma_start(out=xin, in_=x_ap[:, s:s + csz, :])
                # touch: the DMA-completion wait lands here, off the chain
                tch = xinp.tile([128, 1], F32, tag="tch")
                nc.vector.tensor_copy(out=tch, in_=xin[:, 0, :1])
            off = s - c0
            o = s - WARM
            if o >= 0:
                u = upair[(o // 2) % NPAIR][:, o & 1, :]
            else:
                u = upair[0][:, s & 1, :]
            nc.vector.tensor_add(out=u, in0=xin[:, off, :], in1=z)
            nc.vector.scalar_tensor_tensor(out=m, in0=u, scalar=1.0, in1=u,
                                           op0=OP.is_lt, op1=OP.mult)
            nc.vector.scalar_tensor_tensor(out=z, in0=m, scalar=0.0,
                                           in1=w1t2, op0=OP.max,
                                           op1=OP.mult)
            if off == csz - 1 and s < t_in - 1:
                # release marker: DVE becomes the slot's last writer so the
                # refill DMA needs only a single engine-sem wait
                nc.vector.memset(xin[:, 0, :1], 0.0)

            if o >= 0 and (o % 2 == 1 or o == LA - 1):
                op0 = o - (o % 2)
                npair = min(2, LA - op0)
                oo = op0 - o0
                if oo == 0:
                    osz = min(CHUNK, LA - op0)
                    out_tile = outp.tile([128, osz, 2 * C], I8, tag="out")
                    # absorb the out-slot-release (DMA read done) wait
                    nc.scalar.memzero(out_tile[:, 0, 0:4])
                srcp = upair[(op0 // 2) % NPAIR]
                if npair == 2:
                    nc.scalar.activation(out=out_tile[:, oo:oo + 2, :],
                                         in_=srcp, func=AF.Sign,
                                         bias=negone, scale=1.0)
                else:
                    nc.scalar.activation(out=out_tile[:, oo, :],
                                         in_=srcp[:, 0, :], func=AF.Sign,
                                         bias=negone, scale=1.0)
                if oo + npair == osz:
                    # ACT-issued: keeps the SP queue free for input loads
                    nc.scalar.dma_start(out=out_aps[oc], in_=out_tile)
                    o0 += osz
                    oc += 1


def _legalize_waits(nc):
    """Walrus accepts at most one sync wait on compute/DMA ISA structs.
    Split extra waits onto standalone EventSemaphore instructions inserted
    just before, on the same engine queue (identical blocking semantics)."""
    import bass_rust
    skip = ("InstEventSemaphore",)
    for f in nc.m.functions:
        for bb in f.blocks:
            insts = bb.instructions
            k = 0
            while k < len(insts):
                i = insts[k]
                si = i.sync_info
                if (si is not None and si.on_wait and len(si.on_wait) > 1
                        and type(i).__name__ not in skip):
                    waits = list(si.on_wait)
                    for j, w in enumerate(waits[:-1]):
                        ev = mybir.InstEventSemaphore(
                            name=f"{i.name}-evw{j}",
                            engine=i.engine,
                            ins=[], outs=[],
                            sync_info=bass_rust.SyncInfo(
                                on_wait=[w], on_update=[]),
                        )
                        insts.insert(k, ev)
                        k += 1
                    i.sync_info = bass_rust.SyncInfo(
                        on_wait=[waits[-1]], on_update=si.on_update)
                k += 1


def build():
    nc = bass.Bass("TRN2", target_bir_lowering=False, debug=False,
                   enable_asserts=False, num_devices=NCORES)
    xd = nc.dram_tensor("x_dve", [B, WARM + LA, 2 * C], F32,
                        kind="ExternalInput")
    w12_d = nc.dram_tensor("w1b2", [128, 2 * C], F32, kind="ExternalInput")
    out_d = [
        nc.dram_tensor(f"spikes{i}", [B, osz, 2 * C], I8,
                       kind="ExternalOutput")
        for i, osz in enumerate(out_chunk_sizes(LA))
    ]
    with TileContext(nc) as tc:
        lif_body(tc, [d[:] for d in out_d], xd[:], w12_d[:])
    _legalize_waits(nc)
    return nc


def _host_repair(out, x, w1):
    """Exactly recompute lanes whose warmup windows lack a reset/clamp
    certificate at some sub-shard boundary, and patch them into `out`."""
    bounds = []
    for k in range(NCORES):
        if k > 0:
            bounds.append(k * L)
        bounds.append(k * L + LA)
    missing = np.zeros((B, C), bool)
    for t0 in bounds:
        win = x[:, t0 - WARM:t0, :]
        cert = ((win >= np.float32(1.0)) |
                (win <= -w1[None, None, :])).any(axis=1)
        missing |= ~cert
    if not missing.any():
        return 0
    bb, cc = np.nonzero(missing)
    xs = x[bb, :, cc]
    a = w1[cc]
    zz = np.zeros(len(bb), np.float32)
    one = np.float32(1.0)
    zero = np.float32(0.0)
    sp = np.empty((len(bb), T), np.float32)
    for t in range(T):
        u = (xs[:, t] + zz).astype(np.float32)
        mm = ((u < one).astype(np.float32) * u).astype(np.float32)
        zz = (np.maximum(mm, zero) * a).astype(np.float32)
        sp[:, t] = (u > one).astype(np.float32)
    out[bb, :, cc] = sp
    return len(bb)


def kernel(x, w_leak):
    global LAST_RESULTS
    x = np.ascontiguousarray(np.asarray(x), dtype=np.float32)
    w_leak = np.ascontiguousarray(np.asarray(w_leak), dtype=np.float32)
    w1 = (np.float32(1.0) - w_leak).astype(np.float32)
    w1b2 = np.ascontiguousarray(np.broadcast_to(
        np.concatenate([w1, w1])[None, :], (128, 2 * C)), dtype=np.float32)

    # xw: x with a global WARM-zero prefix so every warmup window indexes
    # uniformly (core 0 sub-shard A starts exactly from z=0). One extra
    # zero tail step feeds sub-shard B's dummy final round.
    nsteps = WARM + LA
    xw = np.concatenate([np.zeros((B, WARM, C), np.float32), x,
                         np.zeros((B, 1, C), np.float32)], axis=1)
    in_maps = []
    for k in range(NCORES):
        t0 = k * L
        xa = xw[:, t0:t0 + nsteps, :]
        xb = xw[:, t0 + LA:t0 + LA + nsteps, :]
        xd = np.ascontiguousarray(
            np.stack([xa, xb], axis=2).reshape(B, nsteps, 2 * C))
        in_maps.append({"x_dve": xd, "w1b2": w1b2})

    nc = build()
    res = bass_utils.run_bass_kernel_spmd(
        nc, in_maps, core_ids=list(range(NCORES)), trace=TRACE)
    LAST_RESULTS = res
    nch = len(out_chunk_sizes(LA))
    out = np.empty((B, T, C), np.float32)
    for k in range(NCORES):
        t0 = k * L
        da = np.concatenate(
            [res.results[k][f"spikes{i}"] for i in range(nch)],
            axis=1).reshape(B, LA, 2, C)
        out[:, t0:t0 + LA, :] = da[:, :, 0, :] > 0
        out[:, t0 + LA:t0 + L, :] = da[:, :LA - 1, 1, :] > 0
    _host_repair(out, x, w1)
    return out
